# revision 14
# baseline (speedup 1.0000x reference)
"""Trainium2 Bass kernel: ContinuousNormalizingFlow transform_and_log_det.

Math summary (reference: ReversibleHeun, 10 steps, exact trace):
  - MLP drift f(t,y) = W3 silu(W2 silu(W1 [y;t] + b1) + b2) + b3.
  - Jacobian trace tr(J) = s1'^T M s2' with M = (W1d @ W3) * W2^T (host-folded),
    s1' = silu'(z1), s2' = silu'(z2).
  - ReversibleHeun's yhat leapfrogs (yh_{k+1} = yh_{k-1} + 2dt f_k) and yh is
    only ever consumed through W1d@yh, so the recurrence is kept directly in
    pre-activation space inside PSUM:
        Z1_{k+1} = Z1_{k-1} + c_k (W1d@W3) @ S2_k      (c_0 = dt, else 2dt)
    with S2_k = silu(Z2_k + b2) the layer-2 activations.
  - Outputs collapse to trapezoid sums accumulated in PSUM:
        y_out = y0 + sum_k w_k f_k,   l_out = sum_k w_k tr_k,
        w = [dt/2, dt, ..., dt, dt/2] over the 11 vf evals.
  - All bias constants fold into ACT's per-partition bias
    (b1t[:,k] = b1 + t_k W1[:,16] + k dt (W1d @ b3)); the b3 part of y_out is
    restored on the host.

Layout: feature-major [features, batch], batch 4096 sharded 512/core across
8 cores (pure data parallel). Matmul operands are float32r (single-pass PE).
Phase A (Silu table set) runs the 11 sequential evals pipelined over two
256-sample half-batches, storing S1/S2 into persistent SBUF history; phase B
(Derivative_silu set) replays the cheap matmuls from that history and
computes all traces with no cross-step deps. One ACT table switch total.
"""

import json
import numpy as np
from contextlib import ExitStack

import concourse.bass as bass
import concourse.tile as tile
from concourse import mybir
from concourse.bass_utils import run_bass_kernel_spmd

F32 = mybir.dt.float32
F32R = mybir.dt.float32r
AF = mybir.ActivationFunctionType

D = 16          # data dim
H = 128         # hidden width
B = 4096        # full batch
NCORES = 8
BL = B // NCORES  # 512 per-core batch
HB = BL // 2      # half-batch for phase-A pipelining
NK = 11           # vf evaluations (init + 10 steps)

# packed [128, x] constant column layout (phase-A chunk first)
_C_W2T = (0, 128)
_C_B1T = (128, 139)
_C_B2C = (139, 140)
_C_W13T_1 = (140, 268)
_C_W13T_2 = (268, 396)
_CHUNK1 = 396
_C_MM = (396, 524)
_C_W3T_H = (524, 540)
_C_W3T_F = (540, 556)
_C_ONESW = (556, 558)
_WCOLS = 558
# packed [16, x] layout: w1dT then y0t
_YCOLS = H + BL

# walrus codegen limits sync-waits per instruction: CTRL-class (Drain etc.)
# take only 1; compute/DMA instructions take 2.
_WAIT_LIMIT_DEFAULT = 1
_WAIT_LIMIT = {"Drain": 1, "NoOp": 1, "AllEngineBarrier": 1, "Halt": 1,
               "EventSemaphore": 1}
_SILU_SET_ID = 18  # "silu_and_others" in the compiler act_info.json


def _split_excess_waits(bj: bytes) -> bytes:
    """Hoist excess sync-waits onto inserted same-engine NoOps, and pre-place
    the Silu act-table load at the top of the ACT stream so it hides under
    the initial DMA wait."""
    d = json.loads(bj)
    ctr = 0
    for f in d.get("functions", []):
        for blk in f.get("blocks", []):
            out = []
            for inst in blk.get("instructions", []):
                si = inst.get("sync_info")
                ow = (si or {}).get("on_wait") or []
                lim = _WAIT_LIMIT.get(inst.get("opcode"), _WAIT_LIMIT_DEFAULT)
                if len(ow) > lim:
                    extra = ow[:-lim]
                    for j in range(0, len(extra), 1):
                        ctr += 1
                        out.append({
                            "debug": inst.get("debug"),
                            "engine": inst["engine"],
                            "ins": [], "outs": [],
                            "name": f"I-xw{ctr}",
                            "opcode": "NoOp",
                            "sync_info": {"on_update": [],
                                          "on_wait": extra[j:j + 1]},
                        })
                    si["on_wait"] = ow[-lim:]
                out.append(inst)
            blk["instructions"] = out
    return json.dumps(d).encode()


def _build_nc():
    nc = bass.Bass()

    wpack = nc.dram_tensor("wpack", [H, _WCOLS], F32R, kind="ExternalInput")
    ypack = nc.dram_tensor("ypack", [D, _YCOLS], F32R, kind="ExternalInput")

    yacc_out = nc.dram_tensor("yacc_out", [D, BL], F32, kind="ExternalOutput")
    lacc_out = nc.dram_tensor("lacc_out", [1, BL], F32, kind="ExternalOutput")

    with tile.TileContext(nc) as tc:
        with ExitStack() as ctx:
            singles = ctx.enter_context(tc.tile_pool(name="singles", bufs=1))
            psp = ctx.enter_context(tc.tile_pool(name="psp", bufs=4))
            pout = ctx.enter_context(tc.tile_pool(name="pout", bufs=1))
            pacc = ctx.enter_context(
                tc.tile_pool(name="pacc", bufs=1, space="PSUM"))

            # ---- load packed constants (2 DMAs) ----
            wp = singles.tile([H, _WCOLS], F32R, tag="wp")
            nc.scalar.dma_start(wp[:, 0:_CHUNK1], wpack[:, 0:_CHUNK1])
            nc.gpsimd.dma_start(wp[:, _CHUNK1:_WCOLS],
                                wpack[:, _CHUNK1:_WCOLS])
            yp = singles.tile([D, _YCOLS], F32R, tag="yp")
            nc.sync.dma_start(yp[:], ypack[:])

            cut = lambda c: wp[:, c[0]:c[1]]
            c_w13T_1 = cut(_C_W13T_1)
            c_w13T_2 = cut(_C_W13T_2)
            c_w2T = cut(_C_W2T)
            c_mM = cut(_C_MM)
            c_w3T_h = cut(_C_W3T_H)
            c_w3T_f = cut(_C_W3T_F)
            c_onesw = cut(_C_ONESW)
            b1t_k = lambda k: wp[:, _C_B1T[0] + k:_C_B1T[0] + k + 1].bitcast(F32)
            c_b2c = wp[:, _C_B2C[0]:_C_B2C[1]].bitcast(F32)
            c_w1dT = yp[:, 0:H]
            c_y0t = yp[:, H:H + BL]

            # persistent activation history (written by ACT, read as rhs)
            s1save = singles.tile([H, NK, BL], F32R, tag="s1save")
            s2save = singles.tile([H, NK, BL], F32R, tag="s2save")



            # ---- phase A: forward evals (Silu set), 2 half-batch streams ----
            with tc.tile_pool(name="paccA", bufs=1, space="PSUM") as paccA, \
                    tc.tile_pool(name="pzA", bufs=3, space="PSUM") as pz:
                yacc_ps = paccA.tile([D, BL], F32, tag="yacc")
                z1b = [[paccA.tile([H, HB], F32, name=f"z1_{p}_{h}",
                                   tag=f"z1_{p}_{h}")
                        for h in range(2)] for p in range(2)]
                for p in range(2):
                    for h in range(2):
                        nc.tensor.matmul(
                            z1b[p][h][:], c_w1dT,
                            c_y0t[:, h * HB:(h + 1) * HB],
                            start=True, stop=False)

                for k in range(NK):
                    for h in range(2):
                        hs, he = h * HB, (h + 1) * HB
                        bank = z1b[k % 2][h]
                        s1 = s1save[:, k, hs:he]
                        nc.scalar.activation(s1, bank[:], AF.Silu,
                                             bias=b1t_k(k))
                        z2 = pz.tile([H, HB], F32, tag="z")
                        nc.tensor.matmul(z2[:], c_w2T, s1,
                                         start=True, stop=True)
                        s2 = s2save[:, k, hs:he]
                        nc.scalar.activation(s2, z2[:], AF.Silu, bias=c_b2c)
                        if k < NK - 1:
                            nxt = z1b[(k + 1) % 2][h]
                            nc.tensor.matmul(
                                nxt[:], c_w13T_1 if k == 0 else c_w13T_2,
                                s2, start=False, stop=(k in (8, 9)))
                    nc.tensor.matmul(
                        yacc_ps[:],
                        c_w3T_h if k in (0, NK - 1) else c_w3T_f,
                        s2save[:, k, :], start=(k == 0), stop=(k == NK - 1))

                # y output is complete after phase A: ship it early
                yout = pout.tile([D, BL], F32, tag="yout")
                nc.vector.tensor_copy(yout[:], yacc_ps[:])
                nc.sync.dma_start(yacc_out[:], yout[:])

            # ---- phase B: traces (Derivative_silu set), full FD=512 ----
            with tc.tile_pool(name="paccB", bufs=1, space="PSUM") as paccB, \
                    tc.tile_pool(name="pzB", bufs=5, space="PSUM") as pz:
                lacc_ps = paccB.tile([1, BL], F32, tag="lacc")
                z1f = [paccB.tile([H, BL], F32, name=f"z1f_{p}", tag=f"z1f_{p}")
                       for p in range(2)]
                for p in range(2):
                    nc.tensor.matmul(z1f[p][:], c_w1dT, c_y0t,
                                     start=True, stop=False)

                # ---- phase separator: keep the ACT table sets batched ----
                tc.no_sync_barrier()

                for k in range(NK):
                    bank = z1f[k % 2]
                    sp1 = psp.tile([H, BL], F32R, tag="sp1")
                    nc.scalar.activation(sp1[:], bank[:], AF.Derivative_silu,
                                         bias=b1t_k(k))
                    z2r = pz.tile([H, BL], F32, tag="z")
                    nc.tensor.matmul(z2r[:], c_w2T, s1save[:, k, :],
                                     start=True, stop=True)
                    sp2 = psp.tile([H, BL], F32, tag="sp2")
                    nc.scalar.activation(sp2[:], z2r[:], AF.Derivative_silu,
                                         bias=c_b2c)
                    tm = pz.tile([H, BL], F32, tag="z")
                    nc.tensor.matmul(tm[:], c_mM, sp1[:],
                                     start=True, stop=True)
                    tt = psp.tile([H, BL], F32R, tag="tt")
                    nc.vector.tensor_mul(tt[:], tm[:], sp2[:])
                    nc.tensor.matmul(
                        lacc_ps[:],
                        c_onesw[:, 0:1] if k in (0, NK - 1) else c_onesw[:, 1:2],
                        tt[:], start=(k == 0), stop=(k == NK - 1))
                    if k < NK - 1:
                        nxt = z1f[(k + 1) % 2]
                        nc.tensor.matmul(
                            nxt[:], c_w13T_1 if k == 0 else c_w13T_2,
                            s2save[:, k, :], start=False, stop=(k in (8, 9)))

                lout = pout.tile([1, BL], F32, tag="lout")
                nc.vector.tensor_copy(lout[:], lacc_ps[:])
                nc.sync.dma_start(lacc_out[:], lout[:])

    _orig = nc.to_json_bytes
    nc.to_json_bytes = lambda: _split_excess_waits(_orig())
    return nc


def _host_constants(W1, b1, W2, b2, W3, b3):
    """Fold weights on the host (float64 for accuracy, cast to f32),
    packed into one [128, _WCOLS] tensor."""
    W1f = W1.astype(np.float64)
    W2f = W2.astype(np.float64)
    W3f = W3.astype(np.float64)
    b1f = b1.astype(np.float64)
    b3f = b3.astype(np.float64)

    W1d = W1f[:, :D]          # [128, 16]
    w1t = W1f[:, D]           # [128]
    W13 = W1d @ W3f           # [128, 128]
    M = W13 * W2f.T           # [128, 128]
    Wb = W1d @ b3f            # [128]

    dt = float(np.float32(0.1))
    t_k = [0.0] + [float(np.float32(k) * np.float32(0.1))
                   for k in range(1, NK)]
    b1t = np.stack(
        [b1f + t_k[k] * w1t + k * dt * Wb for k in range(NK)], axis=1)

    wpack = np.zeros((H, _WCOLS), dtype=np.float64)
    wpack[:, _C_W2T[0]:_C_W2T[1]] = W2f.T
    wpack[:, _C_B1T[0]:_C_B1T[1]] = b1t
    wpack[:, _C_B2C[0]] = b2.astype(np.float64)
    wpack[:, _C_W13T_1[0]:_C_W13T_1[1]] = dt * W13.T
    wpack[:, _C_W13T_2[0]:_C_W13T_2[1]] = 2.0 * dt * W13.T
    wpack[:, _C_MM[0]:_C_MM[1]] = M
    wpack[:, _C_W3T_H[0]:_C_W3T_H[1]] = 0.5 * dt * W3f.T
    wpack[:, _C_W3T_F[0]:_C_W3T_F[1]] = dt * W3f.T
    wpack[:, _C_ONESW[0]] = 0.5 * dt
    wpack[:, _C_ONESW[0] + 1] = dt
    return np.ascontiguousarray(wpack.astype(np.float32))


_NC_CACHE = None


def _get_nc():
    global _NC_CACHE
    if _NC_CACHE is None:
        _NC_CACHE = _build_nc()
    return _NC_CACHE


def run(y, W1, b1, W2, b2, W3, b3, **spmd_kwargs):
    wpack = _host_constants(W1, b1, W2, b2, W3, b3)
    y0t_all = y.astype(np.float32).T  # [16, 4096]
    in_maps = []
    for c in range(NCORES):
        yp = np.zeros((D, _YCOLS), dtype=np.float32)
        yp[:, 0:H] = W1.astype(np.float64)[:, :D].T
        yp[:, H:H + BL] = y0t_all[:, c * BL:(c + 1) * BL]
        in_maps.append({"wpack": wpack, "ypack": np.ascontiguousarray(yp)})

    nc = _get_nc()
    res = run_bass_kernel_spmd(nc, in_maps, core_ids=list(range(NCORES)),
                               **spmd_kwargs)

    dt = float(np.float32(0.1))
    sumw = 2 * (0.5 * dt) + (NK - 2) * dt  # == 1.0 (+f32 rounding), in f64
    yacc = np.concatenate([r["yacc_out"].T for r in res.results], axis=0)
    l_out = np.concatenate([r["lacc_out"][0] for r in res.results], axis=0)
    y_out = (y.astype(np.float64) + sumw * b3.astype(np.float64)[None, :]
             + yacc.astype(np.float64))
    return (y_out.astype(np.float32), l_out.astype(np.float32)), res


def kernel(y, W1, b1, W2, b2, W3, b3):
    (y_out, l_out), _ = run(y, W1, b1, W2, b2, W3, b3)
    return y_out, l_out
